# revision 1
# baseline (speedup 1.0000x reference)
# Trainium2 Bass kernel for nn_DifferentiableFeatureLayer.
#
# Math (per reference):
#   bw[b]   = full_series[starts[b]-W : starts[b]+T]            (B, W+T, C)
#   f_mean  = conv(bw, w1)/s1 ; m2 = conv(bw, w2)/s2
#   var2    = conv(bw^2, w2)/s2 - m2^2 ; f_std = sqrt(var2 + 1e-8)
#   out     = concat([x, BN(f_mean), BN(f_std)], -1)            (B, T, 3C)
# where conv is a per-channel sliding window of length W over time and BN
# normalizes per channel over (B, T).
#
# Sharding: by channel — core k owns channels [4k, 4k+4). BN is per channel,
# so every core is fully independent (no collectives). The host extracts the
# B runtime-indexed windows (tiny: 1.25 MB) and passes x through on the host.
#
# Device compute: the sliding window is a banded (Toeplitz) matmul. For
# channel c, pass p in {0,1}:
#   f[b, 128q+r] = sum_p sum_kp T_p[kp, r] * G[kp, b, q+p]
#   T_p[kp, r]   = w[128p + kp - r]  (0 outside [0, 128))   — stationary
#   G[kp, b, j]  = bw[b, 128j + kp]                         — moving
# so each conv is 2 accumulating PE matmuls (K=128, M=128, N=64) per channel.
#
# The mean feature is kept unnormalized (1/s1 folds into the BN affine);
# f_std = sqrt(acc3/s2 - (acc2/s2)^2 + 1e-8) elementwise; BN per channel is
# one fused affine a*f + b with a, b from per-seg (sum, sumsq) reductions.
#
# All inputs arrive in TWO dma_starts (blob1 = G | constants | toeplitz c0/c1,
# blob2 = toeplitz c2/c3); constants are host-replicated across partitions so
# per-partition scalar operands are direct blob column slices.

import numpy as np

import concourse.bass as bass
import concourse.bacc as bacc
import concourse.tile as tile
from concourse import mybir
from concourse.bass_utils import run_bass_kernel_spmd

B, T, C = 16, 512, 32
W = 128
SERIES_LEN = 100000
WIN_MIN, WIN_MAX = 2.0, 64.0
SHARP = 1.0
BN_EPS = 1e-5
STD_EPS = 1e-8

NCORES = 8
CPC = C // NCORES          # channels per core = 4
NSEG = 2 * CPC             # 8 per-core output channels (4 mean + 4 std)
Q = T // 128               # 4 time blocks
NB = B * Q                 # 64 matmul columns
NBT = B * T                # BN population per channel
F32 = mybir.dt.float32
MUL = mybir.AluOpType.mult
ADD = mybir.AluOpType.add
SQRT = mybir.ActivationFunctionType.Sqrt

GW = CPC * B * (Q + 1)     # 320 g columns in blob1
CW = 64                    # constant columns in blob1
TW = 4 * 128               # toeplitz columns per channel (ty, p, r)


def _sigmoid(x):
    out = np.empty_like(x)
    pos = x >= 0
    out[pos] = 1.0 / (1.0 + np.exp(-x[pos]))
    ex = np.exp(x[~pos])
    out[~pos] = ex / (1.0 + ex)
    return out


def _soft_window_weights(raw):
    # (C,) -> (W, C), float64 for host-side accuracy
    win = WIN_MIN + _sigmoid(raw.astype(np.float64)) * (WIN_MAX - WIN_MIN)
    age = np.arange(W, dtype=np.float64)[::-1]
    return _sigmoid(SHARP * (win[None, :] - age[:, None]))


def _toeplitz_pair(wt):
    # wt: (W,) -> (2, 128, 128) band matrices T_p[kp, r] = wt[128p + kp - r]
    kp = np.arange(128)[:, None]
    r = np.arange(128)[None, :]
    out = np.zeros((2, 128, 128), np.float64)
    for p in range(2):
        idx = 128 * p + kp - r
        valid = (idx >= 0) & (idx < W)
        out[p] = np.where(valid, wt[np.clip(idx, 0, W - 1)], 0.0)
    return out


def _build_nc():
    nc = bacc.Bacc("TRN2", target_bir_lowering=False, debug=False,
                   num_devices=NCORES)
    b1_t = nc.dram_tensor("blob1", [128, GW + CW + 2 * TW], F32,
                          kind="ExternalInput")
    b2_t = nc.dram_tensor("blob2", [128, 2 * TW], F32, kind="ExternalInput")
    # out[r, seg, b*Q+q]: fully contiguous per partition for the store DMA
    out_t = nc.dram_tensor("out", [128, NSEG, NB], F32, kind="ExternalOutput")
    b1ap, b2ap, oap = b1_t.ap(), b2_t.ap(), out_t.ap()

    with tile.TileContext(nc) as tc:
        with (
            tc.tile_pool(name="consts", bufs=1) as cpool,
            tc.tile_pool(name="work", bufs=1) as work,
            tc.tile_pool(name="ps1", bufs=2, space="PSUM") as ps1,
            tc.tile_pool(name="ps2", bufs=2, space="PSUM") as ps2,
            tc.tile_pool(name="ps3", bufs=2, space="PSUM") as ps3,
            tc.tile_pool(name="pss", bufs=1, space="PSUM") as pss,
        ):
            ones_c = cpool.tile([128, 1], F32, tag="ones_c")
            nc.vector.memset(ones_c, 1.0)
            ones_r = cpool.tile([1, 128], F32, tag="ones_r")
            nc.vector.memset(ones_r, 1.0)
            # preload the Sqrt activation table while DMAs stream
            e5s = cpool.tile([1, 1], F32, tag="e5s")
            nc.vector.memset(e5s, BN_EPS)
            scr1 = cpool.tile([1, 1], F32, tag="scr1")
            nc.scalar.activation(scr1, e5s, SQRT)

            blob1 = work.tile([128, GW + CW + 2 * TW], F32, tag="blob1")
            nc.sync.dma_start(out=blob1, in_=b1ap)
            blob2 = work.tile([128, 2 * TW], F32, tag="blob2")
            nc.sync.dma_start(out=blob2, in_=b2ap)

            gv = blob1[:, 0:GW].rearrange("p (c b j) -> p c b j", c=CPC, b=B)
            crow = blob1[:, GW:GW + CW]          # partition-replicated consts
            beta_row = crow[0:1, 0:8]
            rcpan_row = crow[0:1, 8:16]
            rcpa2n_row = crow[0:1, 16:24]
            grc_row = crow[0:1, 24:32]
            gam_row = crow[0:1, 32:40]
            eps8b = crow[:, 48:49]               # [128,1] 1e-8
            eps5r = crow[0:1, 49:50]             # [1,1] 1e-5

            def tslice(c, ty, p):
                i = (2 * ty + p) * 128
                if c < 2:
                    base = GW + CW + c * TW
                    return blob1[:, base + i:base + i + 128]
                base = (c - 2) * TW
                return blob2[:, base + i:base + i + 128]

            gsq = work.tile([128, CPC, B, Q + 1], F32, tag="gsq")
            nc.vector.tensor_mul(gsq, gv, gv)

            # ---- per-channel convs + std feature ----
            # fall[:, s, :]: seg s<4: unnormalized f_mean; s>=4: f_std
            fall = work.tile([128, NSEG, NB], F32, tag="fall")
            for c in range(CPC):
                acc1 = ps1.tile([128, NB], F32, tag="acc1")
                nc.tensor.matmul(acc1, tslice(c, 0, 0), gv[:, c, :, 0:Q],
                                 start=True, stop=False)
                nc.tensor.matmul(acc1, tslice(c, 0, 1), gv[:, c, :, 1:Q + 1],
                                 start=False, stop=True)
                acc2 = ps2.tile([128, NB], F32, tag="acc2")
                nc.tensor.matmul(acc2, tslice(c, 1, 0), gv[:, c, :, 0:Q],
                                 start=True, stop=False)
                nc.tensor.matmul(acc2, tslice(c, 1, 1), gv[:, c, :, 1:Q + 1],
                                 start=False, stop=True)
                acc3 = ps3.tile([128, NB], F32, tag="acc3")
                nc.tensor.matmul(acc3, tslice(c, 1, 0), gsq[:, c, :, 0:Q],
                                 start=True, stop=False)
                nc.tensor.matmul(acc3, tslice(c, 1, 1), gsq[:, c, :, 1:Q + 1],
                                 start=False, stop=True)

                # mean feature: raw acc to SBUF (1/s1 folds into BN affine)
                nc.vector.tensor_copy(fall[:, c, :], acc1)

                # std: v = acc3*k - (acc2*k)^2, k = 1/s2 (f_std after sqrt)
                k_ap = crow[:, 40 + c:41 + c]
                m2 = work.tile([128, NB], F32, tag="m2")
                nc.vector.tensor_scalar_mul(m2, acc2, k_ap)
                tt = work.tile([128, NB], F32, tag="tt")
                nc.vector.tensor_mul(tt, m2, m2)
                vseg = fall[:, CPC + c, :]
                nc.vector.tensor_scalar_mul(vseg, acc3, k_ap)
                nc.vector.tensor_sub(vseg, vseg, tt)
            for c in range(CPC):
                nc.scalar.activation(fall[:, CPC + c, :], fall[:, CPC + c, :],
                                     SQRT, bias=eps8b)

            # ---- BN stats: per-seg sums over (r, b, q) ----
            fsq = work.tile([128, NSEG, NB], F32, tag="fsq")
            nc.vector.tensor_mul(fsq, fall, fall)
            pack = work.tile([128, 2 * NSEG], F32, tag="pack")
            nc.vector.reduce_sum(out=pack[:, 0:NSEG], in_=fall,
                                 axis=mybir.AxisListType.X)
            nc.vector.reduce_sum(out=pack[:, NSEG:2 * NSEG], in_=fsq,
                                 axis=mybir.AxisListType.X)
            sums_ps = pss.tile([1, 2 * NSEG], F32, tag="sums")
            nc.tensor.matmul(sums_ps, ones_c, pack, start=True, stop=True)

            # ---- per-seg BN affine: a = grc*rstd, b = beta - mu*gam*rstd
            mu = work.tile([1, NSEG], F32, tag="mu")
            nc.vector.tensor_mul(mu, sums_ps[:, 0:NSEG], rcpan_row)
            msq = work.tile([1, NSEG], F32, tag="msq")
            nc.vector.tensor_mul(msq, sums_ps[:, NSEG:2 * NSEG], rcpa2n_row)
            tmp = work.tile([1, NSEG], F32, tag="tmp")
            nc.vector.tensor_mul(tmp, mu, mu)
            nc.vector.tensor_sub(msq, msq, tmp)          # biased var of f
            sq = work.tile([1, NSEG], F32, tag="sq")
            nc.scalar.activation(sq, msq, SQRT, bias=eps5r)
            rstd = work.tile([1, NSEG], F32, tag="rstd")
            nc.vector.reciprocal(rstd, sq)
            ab = work.tile([1, 2 * NSEG], F32, tag="ab")
            nc.vector.tensor_mul(ab[:, 0:NSEG], rstd, grc_row)
            gr = work.tile([1, NSEG], F32, tag="gr")
            nc.vector.tensor_mul(gr, rstd, gam_row)
            nc.vector.tensor_mul(tmp, mu, gr)
            nc.vector.tensor_sub(ab[:, NSEG:2 * NSEG], beta_row, tmp)

            # broadcast [1, 16] -> [128, 16] via K=1 matmul with ones
            abb_ps = pss.tile([128, 2 * NSEG], F32, tag="abb")
            nc.tensor.matmul(abb_ps, ones_r, ab, start=True, stop=True)
            abb = work.tile([128, 2 * NSEG], F32, tag="abbs")
            nc.vector.tensor_copy(abb, abb_ps)

            # ---- apply affine + store (one contiguous DMA) ----
            for s in range(NSEG):
                nc.vector.tensor_scalar(out=fall[:, s, :], in0=fall[:, s, :],
                                        scalar1=abb[:, s:s + 1],
                                        scalar2=abb[:, NSEG + s:NSEG + s + 1],
                                        op0=MUL, op1=ADD)
            nc.sync.dma_start(out=oap, in_=fall)

    nc.compile()
    return nc


_CACHE = {}


def _get_nc():
    if "nc" not in _CACHE:
        _CACHE["nc"] = _build_nc()
    return _CACHE["nc"]


def _host_prep(inputs):
    fs = np.ascontiguousarray(np.asarray(inputs["full_series"], np.float32))
    idx = np.asarray(inputs["indices"])
    starts = idx[:, 0].astype(np.int64)
    rows = (starts - W)[:, None] + np.arange(W + T)[None, :]
    bw = fs[rows]                                   # (B, 640, C)
    # G[c, kp, b, j] = bw[b, 128j + kp, c]
    G = bw.reshape(B, Q + 1, 128, C).transpose(3, 2, 0, 1)

    w1 = _soft_window_weights(np.asarray(inputs["raw_win_mean"], np.float64))
    w2 = _soft_window_weights(np.asarray(inputs["raw_win_std"], np.float64))
    s1 = w1.sum(axis=0)
    s2 = w2.sum(axis=0)

    gm = np.asarray(inputs["gamma_mean"], np.float64)
    bm = np.asarray(inputs["beta_mean"], np.float64)
    gs = np.asarray(inputs["gamma_std"], np.float64)
    bs = np.asarray(inputs["beta_std"], np.float64)

    in_maps = []
    for k in range(NCORES):
        ch = list(range(CPC * k, CPC * (k + 1)))
        toep = np.zeros((CPC, 2, 2, 128, 128), np.float64)
        for i, cg in enumerate(ch):
            toep[i, 0] = _toeplitz_pair(w1[:, cg])
            toep[i, 1] = _toeplitz_pair(w2[:, cg])
        rcpa = np.concatenate([1.0 / s1[ch], np.ones(CPC)])
        gam = np.concatenate([gm[ch], gs[ch]])
        cst = np.zeros(CW, np.float64)
        cst[0:8] = np.concatenate([bm[ch], bs[ch]])   # beta
        cst[8:16] = rcpa / NBT                        # rcpan
        cst[16:24] = rcpa * rcpa / NBT                # rcpa2n
        cst[24:32] = gam * rcpa                       # grc
        cst[32:40] = gam                              # gam
        cst[40:44] = 1.0 / s2[ch]                     # rcp2
        cst[48] = STD_EPS
        cst[49] = BN_EPS
        # layouts: [kp, ...]
        gpart = G[ch].transpose(1, 0, 2, 3).reshape(128, GW)
        tpart = toep.transpose(3, 0, 1, 2, 4).reshape(128, 4 * TW)
        cpart = np.broadcast_to(cst[None, :], (128, CW))
        blob1 = np.concatenate([gpart, cpart, tpart[:, 0:2 * TW]], axis=1)
        blob2 = tpart[:, 2 * TW:4 * TW]
        in_maps.append(dict(
            blob1=np.ascontiguousarray(blob1, dtype=np.float32),
            blob2=np.ascontiguousarray(blob2, dtype=np.float32),
        ))
    return in_maps


def _assemble(inputs, results):
    x = np.asarray(inputs["x"], np.float32)
    full = np.empty((B, T, 3 * C), np.float32)
    full[:, :, 0:C] = x
    for k in range(NCORES):
        o = results[k]["out"].reshape(128, 2, CPC, B, Q)
        # [r, feat, c, b, q] -> [b, q, r, c, feat] -> [b, t, c, feat]
        arr = o.transpose(3, 4, 0, 2, 1).reshape(B, T, CPC, 2)
        full[:, :, C + CPC * k:C + CPC * (k + 1)] = arr[:, :, :, 0]
        full[:, :, 2 * C + CPC * k:2 * C + CPC * (k + 1)] = arr[:, :, :, 1]
    return full


def run(inputs, trace=False):
    in_maps = _host_prep(inputs)
    nc = _get_nc()
    res = run_bass_kernel_spmd(nc, in_maps, list(range(NCORES)), trace=trace)
    return _assemble(inputs, res.results), res


def kernel(**inputs):
    out, _ = run(inputs)
    return out



# revision 5
# speedup vs baseline: 1.3797x; 1.3797x over previous
# Trainium2 Bass kernel for nn_DifferentiableFeatureLayer.
#
# Math (per reference):
#   bw[b]   = full_series[starts[b]-W : starts[b]+T]            (B, W+T, C)
#   f_mean  = conv(bw, w1)/s1 ; m2 = conv(bw, w2)/s2
#   var2    = conv(bw^2, w2)/s2 - m2^2 ; f_std = sqrt(var2 + 1e-8)
#   out     = concat([x, BN(f_mean), BN(f_std)], -1)            (B, T, 3C)
# where conv is a per-channel sliding window of length W over time and BN
# normalizes per channel over (B, T).
#
# Sharding: by channel — core k owns channels [4k, 4k+4). BN is per channel,
# so every core is fully independent (no collectives). The host extracts the
# B runtime-indexed windows and passes x through on the host.
#
# Device compute: the sliding window is a banded (Toeplitz) matmul; per
# channel c and pass p in {0,1}:
#   f[b, 128q+r] = sum_p sum_kp T_p[kp, r] * G[kp, b, q+p]
# All Toeplitz weights and G ship as bf16 (tolerance 2e-2 allows it): 2x less
# DMA and 4x faster PE than f32. The std-window Toeplitz is pre-scaled by
# 1/s2 on the host so acc2 = m2 and acc3 = E_w[x^2] directly; the mean conv
# stays raw (1/s1 folds into the BN affine consts).
#
# Per-channel accumulators are grouped into [128, 4, 64] PSUM tiles so every
# elementwise/reduce op covers all 4 channels at once. BN stats are summed
# across partitions with an all-ones [128,128] stationary matmul, which also
# replicates them into every partition — the whole BN affine chain then runs
# on [128, 8/16] tiles and the resulting per-seg a/b scalars feed
# tensor_scalar / activation affine ops directly (no broadcast step).
#
# DMA plan (HWDGE desc-gen serializes at ~625 ns per DMA, transfers serialize
# at ~360 GB/s): consts f32 first (tiny), then G + std-Toeplitz (the long
# std pipeline starts earliest), then mean-Toeplitz. Output is bf16, split
# in two DMAs so the first half's descriptor work overlaps the second half's
# compute.

import numpy as np
import ml_dtypes

import concourse.bass as bass
import concourse.bacc as bacc
import concourse.tile as tile
from concourse import mybir
from concourse.bass_utils import run_bass_kernel_spmd

B, T, C = 16, 512, 32
W = 128
SERIES_LEN = 100000
WIN_MIN, WIN_MAX = 2.0, 64.0
SHARP = 1.0
BN_EPS = 1e-5
STD_EPS = 1e-8

NCORES = 8
CPC = C // NCORES          # channels per core = 4
NSEG = 2 * CPC             # 8 per-core output channels (4 mean + 4 std)
Q = T // 128               # 4 time blocks
NB = B * Q                 # 64 matmul columns
NBT = B * T                # BN population per channel
F32 = mybir.dt.float32
BF16 = mybir.dt.bfloat16
MUL = mybir.AluOpType.mult
ADD = mybir.AluOpType.add
SQRT = mybir.ActivationFunctionType.Sqrt
IDENT = mybir.ActivationFunctionType.Identity

GW = CPC * B * (Q + 1)     # 320 g columns
TW = 2 * 128               # toeplitz columns per channel (p, r)
CW = 64                    # constant columns


def _sigmoid(x):
    out = np.empty_like(x)
    pos = x >= 0
    out[pos] = 1.0 / (1.0 + np.exp(-x[pos]))
    ex = np.exp(x[~pos])
    out[~pos] = ex / (1.0 + ex)
    return out


def _soft_window_weights(raw):
    # (C,) -> (W, C), float64 for host-side accuracy
    win = WIN_MIN + _sigmoid(raw.astype(np.float64)) * (WIN_MAX - WIN_MIN)
    age = np.arange(W, dtype=np.float64)[::-1]
    return _sigmoid(SHARP * (win[None, :] - age[:, None]))


def _toeplitz_pair(wt):
    # wt: (W,) -> (2, 128, 128) band matrices T_p[kp, r] = wt[128p + kp - r]
    kp = np.arange(128)[:, None]
    r = np.arange(128)[None, :]
    out = np.zeros((2, 128, 128), np.float64)
    for p in range(2):
        idx = 128 * p + kp - r
        valid = (idx >= 0) & (idx < W)
        out[p] = np.where(valid, wt[np.clip(idx, 0, W - 1)], 0.0)
    return out


def _build_nc():
    nc = bacc.Bacc("TRN2", target_bir_lowering=False, debug=False,
                   num_devices=NCORES)
    cst_t = nc.dram_tensor("cst", [128, CW], F32, kind="ExternalInput")
    # blobA = G | std-toeplitz (pre-scaled by 1/s2); blobB = mean-toeplitz
    ba_t = nc.dram_tensor("blobA", [128, GW + CPC * TW], BF16,
                          kind="ExternalInput")
    bb_t = nc.dram_tensor("blobB", [128, CPC * TW], BF16,
                          kind="ExternalInput")
    out_t = nc.dram_tensor("out", [128, NSEG, NB], BF16, kind="ExternalOutput")
    cap, baap, bbap, oap = cst_t.ap(), ba_t.ap(), bb_t.ap(), out_t.ap()

    with tile.TileContext(nc) as tc:
        with (
            tc.tile_pool(name="consts", bufs=1) as cpool,
            tc.tile_pool(name="work", bufs=1) as work,
            tc.tile_pool(name="ps1", bufs=1, space="PSUM") as ps1,
            tc.tile_pool(name="ps2", bufs=1, space="PSUM") as ps2,
            tc.tile_pool(name="ps3", bufs=1, space="PSUM") as ps3,
            tc.tile_pool(name="pss", bufs=1, space="PSUM") as pss,
        ):
            # ---- input DMAs (consts, then std path, then mean path) ----
            cst = work.tile([128, CW], F32, tag="cst")
            nc.sync.dma_start(out=cst, in_=cap)
            blobA = work.tile([128, GW + CPC * TW], BF16, tag="blobA")
            nc.sync.dma_start(out=blobA, in_=baap)
            blobB = work.tile([128, CPC * TW], BF16, tag="blobB")
            nc.sync.dma_start(out=blobB, in_=bbap)

            # constant slices (host-replicated across partitions)
            beta_ap = cst[:, 0:NSEG]
            rcpan_ap = cst[:, 8:8 + 2 * NSEG]     # rcpan | rcpa2n (16 cols)
            grc_ap = cst[:, 24:24 + NSEG]
            gam_ap = cst[:, 32:32 + NSEG]
            eps5 = cst[:, 40:41]
            eps8 = cst[:, 41:42]

            # all-ones stationary for the partition-sum matmul (Pool engine,
            # runs during the DMA wait)
            ones_mat = cpool.tile([128, 128], F32, tag="ones")
            nc.gpsimd.memset(ones_mat, 1.0)
            # preload the Sqrt activation table while DMAs stream
            e5s = cpool.tile([1, 1], F32, tag="e5s")
            nc.vector.memset(e5s, BN_EPS)
            scr1 = cpool.tile([1, 1], F32, tag="scr1")
            nc.scalar.activation(scr1, e5s, SQRT)

            gv = blobA[:, 0:GW].rearrange("p (c b j) -> p c b j", c=CPC, b=B)

            def t2s(c, p):
                i = GW + (2 * c + p) * 128
                return blobA[:, i:i + 128]

            def t1s(c, p):
                i = (2 * c + p) * 128
                return blobB[:, i:i + 128]

            # ---- squared data for the E[x^2] conv (bf16, DVE) ----
            gsq = work.tile([128, CPC, B, Q + 1], BF16, tag="gsq")
            nc.vector.tensor_mul(gsq, gv, gv)

            # ---- banded matmuls: std first (long pipeline), mean last ----
            acc2 = ps2.tile([128, CPC, NB], F32, tag="acc2")   # m2
            acc3 = ps3.tile([128, CPC, NB], F32, tag="acc3")   # E_w[x^2]
            acc1 = ps1.tile([128, CPC, NB], F32, tag="acc1")   # raw mean conv
            for c in range(CPC):
                nc.tensor.matmul(acc2[:, c, :], t2s(c, 0), gv[:, c, :, 0:Q],
                                 start=True, stop=False)
                nc.tensor.matmul(acc2[:, c, :], t2s(c, 1), gv[:, c, :, 1:Q + 1],
                                 start=False, stop=True)
            for c in range(CPC):
                nc.tensor.matmul(acc3[:, c, :], t2s(c, 0), gsq[:, c, :, 0:Q],
                                 start=True, stop=False)
                nc.tensor.matmul(acc3[:, c, :], t2s(c, 1), gsq[:, c, :, 1:Q + 1],
                                 start=False, stop=True)
            for c in range(CPC):
                nc.tensor.matmul(acc1[:, c, :], t1s(c, 0), gv[:, c, :, 0:Q],
                                 start=True, stop=False)
                nc.tensor.matmul(acc1[:, c, :], t1s(c, 1), gv[:, c, :, 1:Q + 1],
                                 start=False, stop=True)

            # ---- std path: var = acc3 - acc2^2, f_std = sqrt(var + 1e-8) ----
            # squares of PSUM accs go through Act (Square): HW allows only one
            # PSUM operand per vector op, and this offloads the busy DVE
            tt = work.tile([128, CPC * NB], F32, tag="tt")
            nc.scalar.activation(tt, acc2, mybir.ActivationFunctionType.Square)
            var = work.tile([128, CPC, NB], F32, tag="var")
            nc.vector.tensor_sub(var, acc3, tt.rearrange("p (c n) -> p c n",
                                                         c=CPC))
            fstd = work.tile([128, CPC, NB], F32, tag="fstd")
            nc.scalar.activation(fstd, var, SQRT, bias=eps8)

            # ---- BN stats: per-seg sums over (r, b, q) ----
            # pack cols: 0:4 sum f_mean | 4:8 sum f_std | 8:12 sum f_mean^2
            #          | 12:16 sum f_std^2 (= sum var)
            pack = work.tile([128, 2 * NSEG], F32, tag="pack")
            nc.vector.reduce_sum(out=pack[:, 12:16], in_=var,
                                 axis=mybir.AxisListType.X)
            sqm = work.tile([128, CPC, NB], F32, tag="sqm")
            nc.scalar.activation(sqm, acc1, mybir.ActivationFunctionType.Square)
            nc.vector.reduce_sum(out=pack[:, 0:4], in_=acc1,
                                 axis=mybir.AxisListType.X)
            nc.vector.reduce_sum(out=pack[:, 8:12], in_=sqm,
                                 axis=mybir.AxisListType.X)
            nc.vector.reduce_sum(out=pack[:, 4:8], in_=fstd,
                                 axis=mybir.AxisListType.X)
            # partition sums, replicated into every partition by all-ones
            sums = pss.tile([128, 2 * NSEG], F32, tag="sums")
            nc.tensor.matmul(sums, ones_mat, pack, start=True, stop=True)

            # ---- BN affine chain on replicated [128, 8/16] tiles ----
            musq = work.tile([128, 2 * NSEG], F32, tag="musq")
            nc.vector.tensor_mul(musq, sums, rcpan_ap)   # mu | msq
            mu = musq[:, 0:NSEG]
            tt2 = work.tile([128, NSEG], F32, tag="tt2")
            nc.vector.tensor_mul(tt2, mu, mu)
            varb = work.tile([128, NSEG], F32, tag="varb")
            nc.vector.tensor_sub(varb, musq[:, NSEG:2 * NSEG], tt2)
            mug = work.tile([128, NSEG], F32, tag="mug")
            nc.vector.tensor_mul(mug, mu, gam_ap)        # runs during sqrt
            sqv = work.tile([128, NSEG], F32, tag="sqv")
            nc.scalar.activation(sqv, varb, SQRT, bias=eps5)
            rinv = work.tile([128, NSEG], F32, tag="rinv")
            nc.vector.reciprocal(rinv, sqv)
            ab = work.tile([128, 2 * NSEG], F32, tag="ab")
            a_ap = ab[:, 0:NSEG]
            b_ap = ab[:, NSEG:2 * NSEG]
            nc.vector.tensor_mul(a_ap, rinv, grc_ap)
            bt = work.tile([128, NSEG], F32, tag="bt")
            nc.vector.tensor_mul(bt, mug, rinv)
            nc.vector.tensor_sub(b_ap, beta_ap, bt)

            # ---- apply affine + store (std half on Act, mean half on DVE) ----
            outsb = work.tile([128, NSEG, NB], BF16, tag="outsb")
            for c in range(CPC):
                nc.scalar.activation(outsb[:, CPC + c, :], fstd[:, c, :],
                                     IDENT, bias=b_ap[:, CPC + c:CPC + c + 1],
                                     scale=a_ap[:, CPC + c:CPC + c + 1])
            nc.sync.dma_start(out=oap[:, CPC:NSEG, :],
                              in_=outsb[:, CPC:NSEG, :])
            for c in range(CPC):
                nc.vector.tensor_scalar(out=outsb[:, c, :], in0=acc1[:, c, :],
                                        scalar1=a_ap[:, c:c + 1],
                                        scalar2=b_ap[:, c:c + 1],
                                        op0=MUL, op1=ADD)
            nc.sync.dma_start(out=oap[:, 0:CPC, :], in_=outsb[:, 0:CPC, :])

    nc.compile()
    return nc


_CACHE = {}


def _get_nc():
    if "nc" not in _CACHE:
        _CACHE["nc"] = _build_nc()
    return _CACHE["nc"]


def _host_prep(inputs):
    fs = np.ascontiguousarray(np.asarray(inputs["full_series"], np.float32))
    idx = np.asarray(inputs["indices"])
    starts = idx[:, 0].astype(np.int64)
    rows = (starts - W)[:, None] + np.arange(W + T)[None, :]
    bw = fs[rows]                                   # (B, 640, C)
    # G[c, kp, b, j] = bw[b, 128j + kp, c]
    G = bw.reshape(B, Q + 1, 128, C).transpose(3, 2, 0, 1)

    w1 = _soft_window_weights(np.asarray(inputs["raw_win_mean"], np.float64))
    w2 = _soft_window_weights(np.asarray(inputs["raw_win_std"], np.float64))
    s1 = w1.sum(axis=0)
    s2 = w2.sum(axis=0)
    w2s = w2 / s2[None, :]

    gm = np.asarray(inputs["gamma_mean"], np.float64)
    bm = np.asarray(inputs["beta_mean"], np.float64)
    gs = np.asarray(inputs["gamma_std"], np.float64)
    bs = np.asarray(inputs["beta_std"], np.float64)

    in_maps = []
    for k in range(NCORES):
        ch = list(range(CPC * k, CPC * (k + 1)))
        t1 = np.zeros((CPC, 2, 128, 128), np.float64)
        t2 = np.zeros((CPC, 2, 128, 128), np.float64)
        for i, cg in enumerate(ch):
            t1[i] = _toeplitz_pair(w1[:, cg])
            t2[i] = _toeplitz_pair(w2s[:, cg])
        rcpa = np.concatenate([1.0 / s1[ch], np.ones(CPC)])
        gam = np.concatenate([gm[ch], gs[ch]])
        cstv = np.zeros(CW, np.float64)
        cstv[0:8] = np.concatenate([bm[ch], bs[ch]])   # beta
        cstv[8:16] = rcpa / NBT                        # rcpan
        cstv[16:24] = rcpa * rcpa / NBT                # rcpa2n
        cstv[24:32] = gam * rcpa                       # grc
        cstv[32:40] = gam                              # gam
        cstv[40] = BN_EPS
        cstv[41] = STD_EPS
        # layouts: [kp, ...]
        gpart = G[ch].transpose(1, 0, 2, 3).reshape(128, GW)
        t1part = t1.transpose(2, 0, 1, 3).reshape(128, CPC * TW)
        t2part = t2.transpose(2, 0, 1, 3).reshape(128, CPC * TW)
        in_maps.append(dict(
            cst=np.ascontiguousarray(
                np.broadcast_to(cstv[None, :], (128, CW)), dtype=np.float32),
            blobA=np.ascontiguousarray(
                np.concatenate([gpart, t2part], axis=1),
                dtype=ml_dtypes.bfloat16),
            blobB=np.ascontiguousarray(t1part, dtype=ml_dtypes.bfloat16),
        ))
    return in_maps


def _assemble(inputs, results):
    x = np.asarray(inputs["x"], np.float32)
    full = np.empty((B, T, 3 * C), np.float32)
    full[:, :, 0:C] = x
    for k in range(NCORES):
        o = np.asarray(results[k]["out"], dtype=np.float32)
        o = o.reshape(128, 2, CPC, B, Q)
        # [r, feat, c, b, q] -> [b, q, r, c, feat] -> [b, t, c, feat]
        arr = o.transpose(3, 4, 0, 2, 1).reshape(B, T, CPC, 2)
        full[:, :, C + CPC * k:C + CPC * (k + 1)] = arr[:, :, :, 0]
        full[:, :, 2 * C + CPC * k:2 * C + CPC * (k + 1)] = arr[:, :, :, 1]
    return full


def run(inputs, trace=False):
    in_maps = _host_prep(inputs)
    nc = _get_nc()
    res = run_bass_kernel_spmd(nc, in_maps, list(range(NCORES)), trace=trace)
    return _assemble(inputs, res.results), res


def kernel(**inputs):
    out, _ = run(inputs)
    return out


# revision 8
# speedup vs baseline: 1.4403x; 1.0439x over previous
# Trainium2 Bass kernel for nn_DifferentiableFeatureLayer.
#
# Math (per reference):
#   bw[b]   = full_series[starts[b]-W : starts[b]+T]            (B, W+T, C)
#   f_mean  = conv(bw, w1)/s1 ; m2 = conv(bw, w2)/s2
#   var2    = conv(bw^2, w2)/s2 - m2^2 ; f_std = sqrt(var2 + 1e-8)
#   out     = concat([x, BN(f_mean), BN(f_std)], -1)            (B, T, 3C)
# where conv is a per-channel sliding window of length W over time and BN
# normalizes per channel over (B, T).
#
# Sharding: by channel — core k owns channels [4k, 4k+4). BN is per channel,
# so every core is fully independent (no collectives). The host extracts the
# B runtime-indexed windows and passes x through on the host.
#
# Device compute: the sliding window is a banded (Toeplitz) matmul; per
# channel c and pass p in {0,1}:
#   f[b, 128q+r] = sum_p sum_kp T_p[kp, r] * G[kp, b, q+p]
# All Toeplitz weights and G ship as bf16 (tolerance 2e-2 allows it): 2x less
# DMA and 4x faster PE than f32. The std-window Toeplitz is pre-scaled by
# 1/s2 on the host so acc2 = m2 and acc3 = E_w[x^2] directly; the mean conv
# stays raw (1/s1 folds into the BN affine consts).
#
# Per-channel accumulators are grouped into [128, 4, 64] PSUM tiles so every
# elementwise/reduce op covers all 4 channels at once. BN stats are summed
# across partitions with an all-ones [128,128] stationary matmul, which also
# replicates them into every partition — the whole BN affine chain then runs
# on [128, 8/16] tiles and the resulting per-seg a/b scalars feed
# tensor_scalar / activation affine ops directly (no broadcast step).
#
# DMA plan (HWDGE desc-gen serializes at ~625 ns per DMA, transfers serialize
# at ~360 GB/s): consts f32 first (tiny), then G + std-Toeplitz (the long
# std pipeline starts earliest), then mean-Toeplitz. Output is bf16, split
# in two DMAs so the first half's descriptor work overlaps the second half's
# compute.

import numpy as np
import ml_dtypes

import concourse.bass as bass
import concourse.bacc as bacc
import concourse.tile as tile
from concourse import mybir
from concourse.bass_utils import run_bass_kernel_spmd

B, T, C = 16, 512, 32
W = 128
SERIES_LEN = 100000
WIN_MIN, WIN_MAX = 2.0, 64.0
SHARP = 1.0
BN_EPS = 1e-5
STD_EPS = 1e-8

NCORES = 8
CPC = C // NCORES          # channels per core = 4
NSEG = 2 * CPC             # 8 per-core output channels (4 mean + 4 std)
Q = T // 128               # 4 time blocks
NB = B * Q                 # 64 matmul columns
NBT = B * T                # BN population per channel
F32 = mybir.dt.float32
BF16 = mybir.dt.bfloat16
MUL = mybir.AluOpType.mult
ADD = mybir.AluOpType.add
SQRT = mybir.ActivationFunctionType.Sqrt
IDENT = mybir.ActivationFunctionType.Identity

GW = CPC * B * (Q + 1)     # 320 g columns
TW = 2 * 128               # toeplitz columns per channel (p, r)
CW = 64                    # constant columns


def _sigmoid(x):
    out = np.empty_like(x)
    pos = x >= 0
    out[pos] = 1.0 / (1.0 + np.exp(-x[pos]))
    ex = np.exp(x[~pos])
    out[~pos] = ex / (1.0 + ex)
    return out


def _soft_window_weights(raw):
    # (C,) -> (W, C), float64 for host-side accuracy
    win = WIN_MIN + _sigmoid(raw.astype(np.float64)) * (WIN_MAX - WIN_MIN)
    age = np.arange(W, dtype=np.float64)[::-1]
    return _sigmoid(SHARP * (win[None, :] - age[:, None]))


def _toeplitz_pair(wt):
    # wt: (W,) -> (2, 128, 128) band matrices T_p[kp, r] = wt[128p + kp - r]
    kp = np.arange(128)[:, None]
    r = np.arange(128)[None, :]
    out = np.zeros((2, 128, 128), np.float64)
    for p in range(2):
        idx = 128 * p + kp - r
        valid = (idx >= 0) & (idx < W)
        out[p] = np.where(valid, wt[np.clip(idx, 0, W - 1)], 0.0)
    return out


def _build_nc():
    nc = bacc.Bacc("TRN2", target_bir_lowering=False, debug=False,
                   num_devices=NCORES)
    cst_t = nc.dram_tensor("cst", [128, CW], F32, kind="ExternalInput")
    # blobA = G | std-toeplitz (pre-scaled by 1/s2); blobB = mean-toeplitz
    ba_t = nc.dram_tensor("blobA", [128, GW + CPC * TW], BF16,
                          kind="ExternalInput")
    bb_t = nc.dram_tensor("blobB", [128, CPC * TW], BF16,
                          kind="ExternalInput")
    out_t = nc.dram_tensor("out", [128, NSEG, NB], BF16, kind="ExternalOutput")
    cap, baap, bbap, oap = cst_t.ap(), ba_t.ap(), bb_t.ap(), out_t.ap()

    with tile.TileContext(nc) as tc:
        with (
            tc.tile_pool(name="consts", bufs=1) as cpool,
            tc.tile_pool(name="work", bufs=1) as work,
            tc.tile_pool(name="ps1", bufs=1, space="PSUM") as ps1,
            tc.tile_pool(name="ps2", bufs=1, space="PSUM") as ps2,
            tc.tile_pool(name="ps3", bufs=1, space="PSUM") as ps3,
            tc.tile_pool(name="pss", bufs=1, space="PSUM") as pss,
        ):
            # ---- input DMAs: std-path blob first (it gates the longest
            # pipeline), mean toeplitz second; consts go via the Pool-engine
            # SWDGE path so their descriptor gen stays off the HWDGE queue
            blobA = work.tile([128, GW + CPC * TW], BF16, tag="blobA")
            nc.sync.dma_start(out=blobA, in_=baap)
            blobB = work.tile([128, CPC * TW], BF16, tag="blobB")
            nc.sync.dma_start(out=blobB, in_=bbap)
            cst = work.tile([128, CW], F32, tag="cst")
            nc.gpsimd.dma_start(out=cst, in_=cap)

            # constant slices (host-replicated across partitions), grouped
            # per BN half: m = mean segs 0:4, s = std segs 4:8
            rcp_m = cst[:, 0:8]        # rcpan_m | rcpa2n_m
            rcp_s = cst[:, 8:16]
            grc_m, grc_s = cst[:, 16:20], cst[:, 20:24]
            gam_m, gam_s = cst[:, 24:28], cst[:, 28:32]
            beta_m, beta_s = cst[:, 32:36], cst[:, 36:40]
            eps5 = cst[:, 40:41]
            eps8 = cst[:, 41:42]

            # all-ones stationary for the partition-sum matmul (Pool engine,
            # runs during the DMA wait)
            ones_mat = cpool.tile([128, 128], F32, tag="ones")
            nc.gpsimd.memset(ones_mat, 1.0)
            # preload the Sqrt activation table while DMAs stream
            e5s = cpool.tile([1, 1], F32, tag="e5s")
            nc.vector.memset(e5s, BN_EPS)
            scr1 = cpool.tile([1, 1], F32, tag="scr1")
            nc.scalar.activation(scr1, e5s, SQRT)

            gv = blobA[:, 0:GW].rearrange("p (c b j) -> p c b j", c=CPC, b=B)

            def t2s(c, p):
                i = GW + (2 * c + p) * 128
                return blobA[:, i:i + 128]

            def t1s(c, p):
                i = (2 * c + p) * 128
                return blobB[:, i:i + 128]

            # ---- squared data for the E[x^2] conv (bf16, DVE) ----
            gsq = work.tile([128, CPC, B, Q + 1], BF16, tag="gsq")
            nc.vector.tensor_mul(gsq, gv, gv)

            # ---- banded matmuls: std first (long pipeline), mean last ----
            acc2 = ps2.tile([128, CPC, NB], F32, tag="acc2")   # m2
            acc3 = ps3.tile([128, CPC, NB], F32, tag="acc3")   # E_w[x^2]
            acc1 = ps1.tile([128, CPC, NB], F32, tag="acc1")   # raw mean conv
            for c in range(CPC):
                nc.tensor.matmul(acc2[:, c, :], t2s(c, 0), gv[:, c, :, 0:Q],
                                 start=True, stop=False)
                nc.tensor.matmul(acc2[:, c, :], t2s(c, 1), gv[:, c, :, 1:Q + 1],
                                 start=False, stop=True)
            for c in range(CPC):
                nc.tensor.matmul(acc3[:, c, :], t2s(c, 0), gsq[:, c, :, 0:Q],
                                 start=True, stop=False)
                nc.tensor.matmul(acc3[:, c, :], t2s(c, 1), gsq[:, c, :, 1:Q + 1],
                                 start=False, stop=True)
            for c in range(CPC):
                nc.tensor.matmul(acc1[:, c, :], t1s(c, 0), gv[:, c, :, 0:Q],
                                 start=True, stop=False)
                nc.tensor.matmul(acc1[:, c, :], t1s(c, 1), gv[:, c, :, 1:Q + 1],
                                 start=False, stop=True)

            # ---- std path: var = acc3 - acc2^2, f_std = sqrt(var + 1e-8) ----
            # squares of PSUM accs go through Act (Square): HW allows only one
            # PSUM operand per vector op, and this offloads the busy DVE
            tt = work.tile([128, CPC * NB], F32, tag="tt")
            nc.scalar.activation(tt, acc2, mybir.ActivationFunctionType.Square)
            var = work.tile([128, CPC, NB], F32, tag="var")
            nc.vector.tensor_sub(var, acc3, tt.rearrange("p (c n) -> p c n",
                                                         c=CPC))
            fstd = work.tile([128, CPC, NB], F32, tag="fstd")
            nc.scalar.activation(fstd, var, SQRT, bias=eps8)

            # ---- BN stats: per-seg sums over (r, b, q), split per half so
            # the std chain starts as soon as its own two reduces finish
            pack_s = work.tile([128, NSEG], F32, tag="pack_s")
            pack_m = work.tile([128, NSEG], F32, tag="pack_m")
            nc.vector.reduce_sum(out=pack_s[:, 4:8], in_=var,
                                 axis=mybir.AxisListType.X)
            nc.vector.reduce_sum(out=pack_s[:, 0:4], in_=fstd,
                                 axis=mybir.AxisListType.X)
            sqm = work.tile([128, CPC, NB], F32, tag="sqm")
            nc.scalar.activation(sqm, acc1, mybir.ActivationFunctionType.Square)
            # partition sums, replicated into every partition by all-ones
            sums = pss.tile([128, 2 * NSEG], F32, tag="sums")
            sums_s, sums_m = sums[:, 0:NSEG], sums[:, NSEG:2 * NSEG]
            nc.tensor.matmul(sums_s, ones_mat, pack_s, start=True, stop=True)

            # ---- std BN chain on replicated [128, 4/8] tiles (DVE) ----
            musq_s = work.tile([128, NSEG], F32, tag="musq_s")
            nc.vector.tensor_mul(musq_s, sums_s, rcp_s)   # mu_s | msq_s
            mu_s = musq_s[:, 0:4]
            tt2_s = work.tile([128, CPC], F32, tag="tt2_s")
            nc.vector.tensor_mul(tt2_s, mu_s, mu_s)
            varb_s = work.tile([128, CPC], F32, tag="varb_s")
            nc.vector.tensor_sub(varb_s, musq_s[:, 4:8], tt2_s)
            mug_s = work.tile([128, CPC], F32, tag="mug_s")
            nc.vector.tensor_mul(mug_s, mu_s, gam_s)      # runs during sqrt
            sqv_s = work.tile([128, CPC], F32, tag="sqv_s")
            nc.scalar.activation(sqv_s, varb_s, SQRT, bias=eps5)

            # mean-half reduces fill the DVE while Act does the std sqrt
            nc.vector.reduce_sum(out=pack_m[:, 0:4], in_=acc1,
                                 axis=mybir.AxisListType.X)
            nc.vector.reduce_sum(out=pack_m[:, 4:8], in_=sqm,
                                 axis=mybir.AxisListType.X)
            nc.tensor.matmul(sums_m, ones_mat, pack_m, start=True, stop=True)

            ab_s = work.tile([128, NSEG], F32, tag="ab_s")
            a_s, b_s = ab_s[:, 0:4], ab_s[:, 4:8]
            rinv_s = work.tile([128, CPC], F32, tag="rinv_s")
            nc.vector.reciprocal(rinv_s, sqv_s)
            nc.vector.tensor_mul(a_s, rinv_s, grc_s)
            bt_s = work.tile([128, CPC], F32, tag="bt_s")
            nc.vector.tensor_mul(bt_s, mug_s, rinv_s)
            nc.vector.tensor_sub(b_s, beta_s, bt_s)

            # ---- std affine (2 on DVE + 2 on Pool) + store ----
            outsb = work.tile([128, NSEG, NB], BF16, tag="outsb")
            for c in range(2):
                nc.vector.tensor_scalar(out=outsb[:, CPC + c, :],
                                        in0=fstd[:, c, :],
                                        scalar1=a_s[:, c:c + 1],
                                        scalar2=b_s[:, c:c + 1],
                                        op0=MUL, op1=ADD)
            for c in range(2, CPC):
                nc.gpsimd.tensor_scalar(out=outsb[:, CPC + c, :],
                                        in0=fstd[:, c, :],
                                        scalar1=a_s[:, c:c + 1],
                                        scalar2=b_s[:, c:c + 1],
                                        op0=MUL, op1=ADD)
            nc.sync.dma_start(out=oap[:, CPC:NSEG, :],
                              in_=outsb[:, CPC:NSEG, :])

            # ---- mean BN chain (DVE) ----
            musq_m = work.tile([128, NSEG], F32, tag="musq_m")
            nc.vector.tensor_mul(musq_m, sums_m, rcp_m)
            mu_m = musq_m[:, 0:4]
            tt2_m = work.tile([128, CPC], F32, tag="tt2_m")
            nc.vector.tensor_mul(tt2_m, mu_m, mu_m)
            varb_m = work.tile([128, CPC], F32, tag="varb_m")
            nc.vector.tensor_sub(varb_m, musq_m[:, 4:8], tt2_m)
            mug_m = work.tile([128, CPC], F32, tag="mug_m")
            nc.vector.tensor_mul(mug_m, mu_m, gam_m)
            sqv_m = work.tile([128, CPC], F32, tag="sqv_m")
            nc.scalar.activation(sqv_m, varb_m, SQRT, bias=eps5)
            ab_m = work.tile([128, NSEG], F32, tag="ab_m")
            a_m, b_m = ab_m[:, 0:4], ab_m[:, 4:8]
            rinv_m = work.tile([128, CPC], F32, tag="rinv_m")
            nc.vector.reciprocal(rinv_m, sqv_m)
            nc.vector.tensor_mul(a_m, rinv_m, grc_m)
            bt_m = work.tile([128, CPC], F32, tag="bt_m")
            nc.vector.tensor_mul(bt_m, mug_m, rinv_m)
            nc.vector.tensor_sub(b_m, beta_m, bt_m)

            # ---- mean affine (2 on Act + 2 on DVE, reading acc1 PSUM) ----
            for c in range(2):
                nc.scalar.activation(outsb[:, c, :], acc1[:, c, :], IDENT,
                                     bias=b_m[:, c:c + 1],
                                     scale=a_m[:, c:c + 1])
            for c in range(2, CPC):
                nc.vector.tensor_scalar(out=outsb[:, c, :], in0=acc1[:, c, :],
                                        scalar1=a_m[:, c:c + 1],
                                        scalar2=b_m[:, c:c + 1],
                                        op0=MUL, op1=ADD)
            nc.sync.dma_start(out=oap[:, 0:CPC, :], in_=outsb[:, 0:CPC, :])

    nc.compile()
    return nc


_CACHE = {}


def _get_nc():
    if "nc" not in _CACHE:
        _CACHE["nc"] = _build_nc()
    return _CACHE["nc"]


def _host_prep(inputs):
    fs = np.ascontiguousarray(np.asarray(inputs["full_series"], np.float32))
    idx = np.asarray(inputs["indices"])
    starts = idx[:, 0].astype(np.int64)
    rows = (starts - W)[:, None] + np.arange(W + T)[None, :]
    bw = fs[rows]                                   # (B, 640, C)
    # G[c, kp, b, j] = bw[b, 128j + kp, c]
    G = bw.reshape(B, Q + 1, 128, C).transpose(3, 2, 0, 1)

    w1 = _soft_window_weights(np.asarray(inputs["raw_win_mean"], np.float64))
    w2 = _soft_window_weights(np.asarray(inputs["raw_win_std"], np.float64))
    s1 = w1.sum(axis=0)
    s2 = w2.sum(axis=0)
    w2s = w2 / s2[None, :]

    gm = np.asarray(inputs["gamma_mean"], np.float64)
    bm = np.asarray(inputs["beta_mean"], np.float64)
    gs = np.asarray(inputs["gamma_std"], np.float64)
    bs = np.asarray(inputs["beta_std"], np.float64)

    in_maps = []
    for k in range(NCORES):
        ch = list(range(CPC * k, CPC * (k + 1)))
        t1 = np.zeros((CPC, 2, 128, 128), np.float64)
        t2 = np.zeros((CPC, 2, 128, 128), np.float64)
        for i, cg in enumerate(ch):
            t1[i] = _toeplitz_pair(w1[:, cg])
            t2[i] = _toeplitz_pair(w2s[:, cg])
        rcpa_m, rcpa_s = 1.0 / s1[ch], np.ones(CPC)
        cstv = np.zeros(CW, np.float64)
        cstv[0:4] = rcpa_m / NBT                       # rcpan_m
        cstv[4:8] = rcpa_m * rcpa_m / NBT              # rcpa2n_m
        cstv[8:12] = rcpa_s / NBT                      # rcpan_s
        cstv[12:16] = rcpa_s * rcpa_s / NBT            # rcpa2n_s
        cstv[16:20] = gm[ch] * rcpa_m                  # grc_m
        cstv[20:24] = gs[ch] * rcpa_s                  # grc_s
        cstv[24:28] = gm[ch]                           # gam_m
        cstv[28:32] = gs[ch]                           # gam_s
        cstv[32:36] = bm[ch]                           # beta_m
        cstv[36:40] = bs[ch]                           # beta_s
        cstv[40] = BN_EPS
        cstv[41] = STD_EPS
        # layouts: [kp, ...]
        gpart = G[ch].transpose(1, 0, 2, 3).reshape(128, GW)
        t1part = t1.transpose(2, 0, 1, 3).reshape(128, CPC * TW)
        t2part = t2.transpose(2, 0, 1, 3).reshape(128, CPC * TW)
        in_maps.append(dict(
            cst=np.ascontiguousarray(
                np.broadcast_to(cstv[None, :], (128, CW)), dtype=np.float32),
            blobA=np.ascontiguousarray(
                np.concatenate([gpart, t2part], axis=1),
                dtype=ml_dtypes.bfloat16),
            blobB=np.ascontiguousarray(t1part, dtype=ml_dtypes.bfloat16),
        ))
    return in_maps


def _assemble(inputs, results):
    x = np.asarray(inputs["x"], np.float32)
    full = np.empty((B, T, 3 * C), np.float32)
    full[:, :, 0:C] = x
    for k in range(NCORES):
        o = np.asarray(results[k]["out"], dtype=np.float32)
        o = o.reshape(128, 2, CPC, B, Q)
        # [r, feat, c, b, q] -> [b, q, r, c, feat] -> [b, t, c, feat]
        arr = o.transpose(3, 4, 0, 2, 1).reshape(B, T, CPC, 2)
        full[:, :, C + CPC * k:C + CPC * (k + 1)] = arr[:, :, :, 0]
        full[:, :, 2 * C + CPC * k:2 * C + CPC * (k + 1)] = arr[:, :, :, 1]
    return full


def run(inputs, trace=False):
    in_maps = _host_prep(inputs)
    nc = _get_nc()
    res = run_bass_kernel_spmd(nc, in_maps, list(range(NCORES)), trace=trace)
    return _assemble(inputs, res.results), res


def kernel(**inputs):
    out, _ = run(inputs)
    return out
